# revision 20
# baseline (speedup 1.0000x reference)
"""Grouped-experts SwiGLU MoE kernel for Trainium2 (8 NeuronCores).

Problem: x [8192, 2048] f32, 8 experts with w1/w3 [8, 1408, 2048] and
w2 [8, 2048, 1408]; tokens are expert-contiguous with a per-expert count
vector. out[t] = (silu(x_t @ w1_e.T) * (x_t @ w3_e.T)) @ w2_e.T for the
expert e owning token t.

Sharding: pure expert parallelism. Core e receives expert e's 1024-token
tile (dynamic-slice semantics of the reference) plus expert e's weights,
and computes the full SwiGLU MLP for that tile. No collectives.

Performance structure (PE-bound problem: 1056 matmuls x 512 free-dim
~= 225 us/core at 2.4 GHz; the shared axon trn2 sustains ~2.0 GHz under
8-core load (P0 power state), so the matmul-stream floor is ~270 us.
HW-measured ~290 us interleaved vs ~298 us for the prior structure):
  - all streamed operands are fp16 (quantized host-side, rel err
    ~5e-4 vs the 2e-2 gate): same 1 cycle/row PE rate as f32r but half
    the HBM traffic, so DMA hides completely under PE work.
  - same-PSUM-bank matmul runs: consecutive matmuls that target
    different PSUM banks cost ~20 ns extra each (HW-probed; sem incs,
    satisfied waits, and extra LDWEIGHTS are all free). Both stages
    run each accumulation group as one same-bank k-loop (16 resp. 11
    matmuls per run), with a per-k LDWEIGHTS hidden by pull-ahead.
    ht=0 interleaves its 4 groups in 4-k blocks instead so consumption
    stays paced with the streaming x batches.
  - w1/w3 are packed k-interleaved into one DRAM tensor (one dma_start
    per h-tile; each dma_start costs ~1.26 us of HWDGE SEQ issue).
    x streams in ramped k-batches on the SP queue.
  - the Tile scheduler hoists dependency-free DMAs to the program head
    and the SDMA pool serializes transfers, so deferred loads are
    gated by data deps: the w13 double-buffer WAR rotation defers pair
    ht+1 to iter ht-1, and dummy one-element copies gate pair1 (on the
    first x slice) and each w2 tile (on a mid-stage-1 h-tile output),
    keeping the early SDMA window clear for x.
  - contraction dims (D for stage 1, H for stage 2) live on SBUF
    partitions; all tensors are packed [p, ktile, free] in DRAM so
    every DMA is a contiguous partition-row load and the matmuls need
    no on-device transposes.
  - a post-compile BIR pass (_dedup_ldweights) drops back-to-back
    redundant LDWEIGHTS that Tile emits for matmuls sharing a
    stationary operand.

Stage 1 computes hT [H, T] = silu(w1 xT) * (w3 xT) per 128-row h-tile
(PSUM [128h, 512t] x2 token blocks, contraction over 16 D-tiles);
stage 2 computes out [T, D] db-outer/k-inner (PSUM [128t, 512d],
contraction over 11 H-tiles), each (tt, db) group draining its copy+DMA
while the next accumulates — the end-of-kernel tail is one dim-block.

_build_nc(reps=R, hw_loop=True) wraps the body in a device-side
tc.For_i loop for the timing protocol in test.py (constant NEFF size),
4 executions per iteration (software-pipelined: each later rep's input
DMAs overlap the previous rep's stage-2 under buffer-WAR gating, and
the separate 4+4 PSUM rotations per stage keep the PE seam stall-free
— HW-measured ~2.3 us/rep per halving of the barrier+head count).
"""

from contextlib import ExitStack

import numpy as np

import concourse.bass as bass
import concourse.mybir as mybir
import concourse.tile as tile
from concourse import bacc
from concourse.bass import ts
from concourse.bass_utils import run_bass_kernel_spmd

F32 = mybir.dt.float32
F16 = mybir.dt.float16

N_TOKENS = 8192
DIM = 2048
HIDDEN = 1408
N_EXPERTS = 8
CAP = N_TOKENS // N_EXPERTS  # 1024 tokens per core
P = 128
KD = DIM // P  # 16 contraction tiles, stage 1
KH = HIDDEN // P  # 11 contraction tiles, stage 2
TB = 512  # token-block (stage-1 moving free dim)
DB = 512  # dim-block (stage-2 moving free dim)
N_TB = CAP // TB  # 2
N_DB = DIM // DB  # 4
N_TT = CAP // P  # 8 token tiles (stage-2 stationary)

_CACHED_NC = None


def _build_nc(reps=1, hw_loop=False, unroll=4):
    nc = bacc.Bacc("TRN2", debug=False)
    xQ = nc.dram_tensor("xQ", [P, KD, CAP], F16, kind="ExternalInput").ap()
    w13Q = nc.dram_tensor("w13Q", [KH, P, KD, 2, P], F16, kind="ExternalInput").ap()
    w2Q = nc.dram_tensor("w2Q", [N_DB, P, KH, DB], F16, kind="ExternalInput").ap()
    out = nc.dram_tensor("out", [CAP, DIM], F32, kind="ExternalOutput").ap()

    with tile.TileContext(nc) as tc, ExitStack() as ctx:
        xpool = ctx.enter_context(tc.tile_pool(name="xpool", bufs=1))
        hpool = ctx.enter_context(tc.tile_pool(name="hpool", bufs=1))
        wpool = ctx.enter_context(tc.tile_pool(name="wpool", bufs=2))
        w2pool = ctx.enter_context(tc.tile_pool(name="w2pool", bufs=N_DB))
        tmppool = ctx.enter_context(tc.tile_pool(name="tmppool", bufs=3))
        opool = ctx.enter_context(tc.tile_pool(name="opool", bufs=4))
        # Separate 4-bank PSUM rotations per stage: stage-1 of rep i+1 then
        # reuses banks drained early in rep i's stage 1 (not rep i's last
        # stage-2 outputs), so back-to-back reps have no PSUM WAR stall at
        # the seam.
        ps1pool = ctx.enter_context(tc.tile_pool(name="ps1pool", bufs=4, space="PSUM"))
        ps2pool = ctx.enter_context(tc.tile_pool(name="ps2pool", bufs=4, space="PSUM"))

        def one_rep(streaming=True):
            # streaming=True: post-barrier pack leader — x arrives in
            # ramped k-batches and ht=0 interleaves its groups in 4-k
            # blocks to stay paced with the stream. streaming=False:
            # follower rep in an unrolled pack — its loads (gated by
            # buffer WAR) completed during the previous rep's stage 2,
            # so x loads as one bulk DMA and ht=0 runs plain 16-k
            # same-bank runs like every other h-tile.
            x_sb = xpool.tile([P, KD, CAP], F16)
            # hT tiles: [h-inner(part), h-tile, t]
            h_sb = hpool.tile([P, KH, CAP], F16)

            w13_t = []

            def load_pair(ht):
                # w1+w3 packed k-interleaved in one DRAM tensor: a single
                # dma_start per h-tile (vs 2) halves ACT-queue issue cost
                # and w3 k-tiles land together with their w1 partner.
                w13_sb = wpool.tile([P, KD, 2, P], F16, tag="w13")
                nc.scalar.dma_start(w13_sb[:], w13Q[ht])
                w13_t.append(w13_sb)

            # Queue split: weights stream on the ACT HWDGE queue, x batches
            # and output tiles on the SP queue. The Tile scheduler hoists
            # every dependency-free DMA to the program head, and the SDMA
            # pool serializes transfers — so every deferred load below is
            # gated by a data dependency (buffer-rotation WAR or a dummy
            # one-element copy), not by emission position.
            w13_sb0 = wpool.tile([P, KD, 2, P], F16, tag="w13")
            w13_t.append(w13_sb0)
            if streaming:
                ck = [(0, 2), (2, 2), (4, 4), (8, 8)]  # (start, len) k-chunks
                for c0, cl in ck:
                    nc.scalar.dma_start(
                        w13_sb0[:, c0 : c0 + cl], w13Q[0, :, c0 : c0 + cl]
                    )
                # x in ramped batches: fine-grained while the PE is cold
                # (the ht=0 k-stream), bulk later. Batched because each
                # dma_start costs ~1.26 us of SEQ issue time; 16 single
                # slices can't keep up with warm PE consumption.
                xb = [(0, 1), (1, 1), (2, 2), (4, 2), (6, 2), (8, 4), (12, 2), (14, 2)]
                for b0, bl in xb:
                    nc.sync.dma_start(x_sb[:, b0 : b0 + bl], xQ[:, b0 : b0 + bl])
            else:
                nc.scalar.dma_start(w13_sb0[:], w13Q[0])
                nc.sync.dma_start(x_sb[:], xQ[:])
            # pair1 gated on x batch (0,1): its 1 MB transfer starts right
            # after the first x slice lands, ahead of ht=1's first use.
            w13_sb1 = wpool.tile([P, KD, 2, P], F16, tag="w13")
            nc.vector.tensor_copy(w13_sb1[0:1, 0, 0, 0:1], x_sb[0:1, 0, 0:1])
            nc.scalar.dma_start(w13_sb1[:], w13Q[1])
            w13_t.append(w13_sb1)

            # Stage 1: per h-tile, k-streamed accumulation over 4 PSUM banks
            # (ps1/ps3 x 2 token blocks); banks rotate 8-wide across tiles.
            for ht in range(KH):
                # pair prefetch, double-buffered: the wpool WAR rotation
                # gates pair ht+1's DMA on iter ht-1 releasing the buffer,
                # so the transfer fires one full h-tile span before use.
                if 1 <= ht <= KH - 2:
                    load_pair(ht + 1)
                if ht == 0:
                    w2_t = []
                if ht in (2, 4, 6, 8):
                    # w2 tiles for stage 2: dummy-gated on the previous
                    # h-tile's output so the 1.44 MB transfers spread across
                    # mid-stage-1 instead of hoisting into the x window.
                    db = (ht - 2) // 2
                    w2_sb = w2pool.tile([P, KH, DB], F16, tag="w2")
                    nc.vector.tensor_copy(
                        w2_sb[0:1, 0, 0:1], h_sb[0:1, ht - 1, 0:1]
                    )
                    nc.scalar.dma_start(w2_sb[:], w2Q[db])
                    w2_t.append(w2_sb)
                w13_sb = w13_t[ht]
                # Same-PSUM-bank matmul runs: consecutive matmuls that hit
                # different PSUM banks pay ~20 ns each (HW-measured), so the
                # 4 accumulation groups run k-sequentially, one bank at a
                # time (LDWEIGHTS per k is free — hidden by pull-ahead).
                # ht=0 interleaves in 4-k blocks instead, so consumption
                # stays paced with the streaming x batches.
                kb = 4 if (ht == 0 and streaming) else KD
                ps1 = [
                    ps1pool.tile([P, TB], F32, tag="ps1", name=f"ps1_{ht}_{tb}")
                    for tb in range(N_TB)
                ]
                ps3 = [
                    ps1pool.tile([P, TB], F32, tag="ps1", name=f"ps3_{ht}_{tb}")
                    for tb in range(N_TB)
                ]
                for k0 in range(0, KD, kb):
                    for psg, w_idx, tb in (
                        (ps1[0], 0, 0), (ps3[0], 1, 0),
                        (ps1[1], 0, 1), (ps3[1], 1, 1),
                    ):
                        for k in range(k0, k0 + kb):
                            nc.tensor.matmul(
                                psg[:], w13_sb[:, k, w_idx],
                                x_sb[:, k, ts(tb, TB)],
                                start=(k == 0), stop=(k == KD - 1),
                            )
                for tb in range(N_TB):
                    sil = tmppool.tile([P, TB], F32, tag="sil")
                    nc.scalar.activation(
                        sil[:], ps1[tb][:], mybir.ActivationFunctionType.Silu
                    )
                    nc.vector.tensor_mul(
                        h_sb[:, ht, ts(tb, TB)], sil[:], ps3[tb][:]
                    )

            # Stage 2: out = hT.T @ w2.T — stationary hT token-tiles,
            # moving w2 dim-blocks. db-outer/k-inner: each (tt, db) PSUM
            # group is an 11-matmul same-bank run (no per-MM bank-switch
            # cost; the per-k stationary reloads are free), and each
            # group's copy+DMA drains while the next accumulates — which
            # also shrinks the end-of-kernel drain tail to one dim-block.
            for tt in range(N_TT):
                for db in range(N_DB):
                    ps2 = ps2pool.tile([P, DB], F32, tag="ps2",
                                       name=f"ps2_{tt}_{db}")
                    for k in range(KH):
                        nc.tensor.matmul(
                            ps2[:], h_sb[:, k, ts(tt, P)], w2_t[db][:, k],
                            start=(k == 0), stop=(k == KH - 1),
                        )
                    ot = opool.tile([P, DB], F32, tag="ot",
                                    name=f"ot_{tt}_{db}")
                    nc.vector.tensor_copy(ot[:], ps2[:])
                    nc.sync.dma_start(out[ts(tt, P), ts(db, DB)], ot[:])

        if hw_loop and reps > 1:
            # constant-size NEFF, `unroll` executions per For_i iteration:
            # each later rep's input DMAs (gated only by buffer WAR)
            # overlap the previous rep's stage-2 compute, and the PE
            # crosses the intra-iteration seams without the all-engine
            # barrier or a cold DMA head — the barrier + head cost is
            # paid once per `unroll` executions.
            assert reps % unroll == 0, "reps must divide by unroll"
            with tc.For_i(0, reps // unroll):
                for j in range(unroll):
                    one_rep(streaming=(j == 0))
        else:
            for _ in range(reps):
                one_rep()

    nc.compile()
    _dedup_ldweights(nc)
    return nc


def _dedup_ldweights(nc):
    """Drop back-to-back redundant LDWEIGHTS in the tile-lowered BIR.

    Tile's lowering emits one InstLdweights per InstMatmult even when
    consecutive matmuls share the stationary operand (the PE array keeps
    weights across matmuls with ldweights=false, so the reload is pure
    overhead — measured ~67-90 ns serialized per load). Runs after
    bacc.compile() (post move_matmul_waits_to_ldweights). A duplicate is
    removed only when (a) its operand AP is byte-identical to the
    surviving PE weight load with only Matmult(ldweights=false)/
    EventSemaphore in between on the PE stream, (b) it carries no
    semaphore updates, and (c) its waits are a subset of waits already
    observed on the PE stream since that load (sems are monotonic, so
    the condition is already guaranteed in the in-order PE stream).
    """
    import orjson

    j = orjson.loads(mybir.module_to_json_string(nc.m))
    removed = 0
    for fn in j["functions"]:
        for blk in fn["blocks"]:
            insts = blk.get("instructions", [])
            keep = []
            last_sig = None
            last_waits = []
            for inst in insts:
                if inst.get("engine") != "PE":
                    keep.append(inst)
                    continue
                op = inst.get("opcode")
                if op == "Ldweights":
                    sig = orjson.dumps(
                        [
                            inst.get("ins"),
                            inst.get("tile_position"),
                            inst.get("tile_size"),
                            inst.get("perf_mode"),
                            inst.get("is_transpose"),
                        ]
                    )
                    si = inst.get("sync_info") or {}
                    waits = [orjson.dumps(w) for w in (si.get("on_wait") or [])]
                    if (
                        sig == last_sig
                        and not si.get("on_update")
                        and all(w in last_waits for w in waits)
                    ):
                        removed += 1
                        continue
                    last_sig = sig
                    last_waits = waits
                elif op == "Matmult":
                    if inst.get("ldweights"):
                        last_sig = None  # self-loading matmul clobbers
                    else:
                        si = inst.get("sync_info") or {}
                        last_waits += [
                            orjson.dumps(w) for w in (si.get("on_wait") or [])
                        ]
                elif op != "EventSemaphore":
                    last_sig = None  # unknown PE op: be conservative
                keep.append(inst)
            blk["instructions"] = keep
    if removed:
        nc.m = mybir.module_from_json_bytes(orjson.dumps(j))
    return removed


def _strip_mm_incs_json(j, keep_every=10**9):
    """Strip per-Matmult semaphore increments from the lowered BIR.

    Tile emits `then_inc(PE_sem, 1)` on every Matmult; the EVT_SEM register
    write serializes on the PE at ~26 ns each (see tensor-engine tail
    model), which is pure overhead for the ~93% of matmuls no consumer
    waits on. This keeps an increment only where some wait threshold is
    crossed (plus every `keep_every`-th as padding) and renumbers every
    `sem-ge-imm` wait on those sems to the new cumulative counts. Release
    points are preserved instruction-exactly, so schedule semantics (and
    deadlock-freedom) are unchanged. UpdateValue must stay 1 (BIR ISA).
    """
    mm_sems = set()
    for fn in j["functions"]:
        for blk in fn["blocks"]:
            for inst in blk.get("instructions", []):
                if inst.get("engine") == "PE" and inst.get("opcode") == "Matmult":
                    si = inst.get("sync_info") or {}
                    for u in (si.get("on_update") or []):
                        if u.get("sync_type") == "semaphore":
                            assert u["update_mode"] == "sem-inc"
                            assert u["update_value"] == 1
                            mm_sems.add(u["id"])
    if not mm_sems:
        return 0, 0
    waited = {s: set() for s in mm_sems}
    for fn in j["functions"]:
        for blk in fn["blocks"]:
            for inst in blk.get("instructions", []):
                si = inst.get("sync_info") or {}
                for w in (si.get("on_wait") or []):
                    if w.get("id") in waited:
                        assert w["wait_mode"] == "sem-ge-imm", w
                        waited[w["id"]].add(w["wait_value"])
    val_map = {s: {0: 0} for s in mm_sems}
    cum_old = {s: 0 for s in mm_sems}
    cum_new = {s: 0 for s in mm_sems}
    affine = []  # barrier add/sub-imm updates carrying the per-iter total
    stripped = kept = 0
    for fn in j["functions"]:
        for blk in fn["blocks"]:
            for inst in blk.get("instructions", []):
                si = inst.get("sync_info") or {}
                upd = si.get("on_update") or []
                if not upd:
                    continue
                is_mm = (
                    inst.get("engine") == "PE"
                    and inst.get("opcode") == "Matmult"
                )
                new_upd = []
                for u in upd:
                    s = u.get("id")
                    if u.get("sync_type") != "semaphore" or s not in mm_sems:
                        new_upd.append(u)
                        continue
                    if not is_mm:
                        # For_i barrier bias/reset: carries the per-iteration
                        # inc total; rewritten to the new total below.
                        assert u["update_mode"] in ("sem-add-imm", "sem-sub-imm"), u
                        affine.append((s, u))
                        new_upd.append(u)
                        continue
                    cum_old[s] += 1
                    if cum_old[s] in waited[s] or cum_old[s] % keep_every == 0:
                        cum_new[s] += 1
                        val_map[s][cum_old[s]] = cum_new[s]
                        new_upd.append(u)
                        kept += 1
                    else:
                        stripped += 1
                si["on_update"] = new_upd
    for s, u in affine:
        assert u["update_value"] == cum_old[s], (u, cum_old[s])
        u["update_value"] = cum_new[s]
    for fn in j["functions"]:
        for blk in fn["blocks"]:
            for inst in blk.get("instructions", []):
                si = inst.get("sync_info") or {}
                for w in (si.get("on_wait") or []):
                    s = w.get("id")
                    if s in mm_sems:
                        w["wait_value"] = val_map[s][w["wait_value"]]
    return stripped, kept


def _get_nc():
    global _CACHED_NC
    if _CACHED_NC is None:
        _CACHED_NC = _build_nc()
    return _CACHED_NC


def _pack_inputs(x, w1, w2, w3, read_starts):
    """Per-core input dicts with DMA-optimal (partition-major) layouts."""
    in_maps = []
    for e in range(N_EXPERTS):
        s = int(read_starts[e])
        xe = x[s : s + CAP]  # [CAP, DIM]
        xQ = np.ascontiguousarray(
            xe.T.reshape(KD, P, CAP).transpose(1, 0, 2)
        ).astype(np.float16)
        w1Q = w1[e].T.reshape(KD, P, KH, P).transpose(2, 1, 0, 3)
        w3Q = w3[e].T.reshape(KD, P, KH, P).transpose(2, 1, 0, 3)
        # k-interleaved pack: [KH, P, KD, {w1,w3}, P] — one DMA per h-tile
        w13Q = np.ascontiguousarray(
            np.stack([w1Q, w3Q], axis=3)
        ).astype(np.float16)
        w2Q = np.ascontiguousarray(
            w2[e].T.reshape(KH, P, N_DB, DB).transpose(2, 1, 0, 3)
        ).astype(np.float16)
        in_maps.append({"xQ": xQ, "w13Q": w13Q, "w2Q": w2Q})
    return in_maps


def kernel(x, num_tokens_per_expert, w1, w2, w3):
    x = np.ascontiguousarray(np.asarray(x, dtype=np.float32))
    w1 = np.asarray(w1, dtype=np.float32)
    w2 = np.asarray(w2, dtype=np.float32)
    w3 = np.asarray(w3, dtype=np.float32)
    counts = np.asarray(num_tokens_per_expert).astype(np.int64)

    offsets = np.cumsum(counts)
    starts = offsets - counts
    # jax.lax.dynamic_slice clamps the read start so the slice is in-bounds.
    read_starts = np.clip(starts, 0, N_TOKENS - CAP)

    in_maps = _pack_inputs(x, w1, w2, w3, read_starts)
    nc = _get_nc()
    res = run_bass_kernel_spmd(nc, in_maps, core_ids=list(range(N_EXPERTS)))
    ye = [res.results[e]["out"] for e in range(N_EXPERTS)]

    if np.all(counts == CAP):
        # balanced routing: per-expert tiles are disjoint and exactly cover x
        return np.concatenate(ye, axis=0)

    # general case: mask invalid slots, scatter-add to clipped positions
    y = np.zeros((N_TOKENS, DIM), np.float32)
    slot = np.arange(CAP)
    for e in range(N_EXPERTS):
        valid = slot < counts[e]
        pos = np.clip(starts[e] + slot, 0, N_TOKENS - 1)
        np.add.at(y, pos, np.where(valid[:, None], ye[e], 0.0))
    return y



# revision 23
# speedup vs baseline: 1.0845x; 1.0845x over previous
"""Grouped-experts SwiGLU MoE kernel for Trainium2 (8 NeuronCores).

Problem: x [8192, 2048] f32, 8 experts with w1/w3 [8, 1408, 2048] and
w2 [8, 2048, 1408]; tokens are expert-contiguous with a per-expert count
vector. out[t] = (silu(x_t @ w1_e.T) * (x_t @ w3_e.T)) @ w2_e.T for the
expert e owning token t.

Sharding: pure expert parallelism. Core e receives expert e's 1024-token
tile (dynamic-slice semantics of the reference) plus expert e's weights,
and computes the full SwiGLU MLP for that tile. No collectives.

Performance structure (PE-bound problem: 1056 matmuls x 512 free-dim
~= 225 us/core at 2.4 GHz; the shared axon trn2 sustains ~2.0 GHz under
8-core load (P0 power state), so the matmul-stream floor is ~270 us.
HW-measured ~282 us interleaved; ~298 us for the pre-tune structure):
  - all streamed operands are fp16 (quantized host-side, rel err
    ~5e-4 vs the 2e-2 gate): same 1 cycle/row PE rate as f32r but half
    the HBM traffic, so DMA hides completely under PE work.
  - same-PSUM-bank matmul runs: consecutive matmuls that target
    different PSUM banks cost ~20 ns extra each (HW-probed; sem incs,
    satisfied waits, and extra LDWEIGHTS are all free). Both stages
    run each accumulation group as one same-bank k-loop (16 resp. 11
    matmuls per run), with a per-k LDWEIGHTS hidden by pull-ahead.
    ht=0 interleaves its 4 groups in 4-k blocks instead so consumption
    stays paced with the streaming x batches.
  - w1/w3 are packed k-interleaved into one DRAM tensor (one dma_start
    per h-tile; each dma_start costs ~1.26 us of HWDGE SEQ issue).
    x streams in ramped k-batches on the SP queue.
  - the Tile scheduler hoists dependency-free DMAs to the program head
    and the SDMA pool serializes transfers, so deferred loads are
    gated by data deps: the w13 double-buffer WAR rotation defers pair
    ht+1 to iter ht-1, and dummy one-element copies gate pair1 (on the
    first x slice) and each w2 tile (on a mid-stage-1 h-tile output),
    keeping the early SDMA window clear for x.
  - contraction dims (D for stage 1, H for stage 2) live on SBUF
    partitions; all tensors are packed [p, ktile, free] in DRAM so
    every DMA is a contiguous partition-row load and the matmuls need
    no on-device transposes.
  - a post-compile BIR pass (_dedup_ldweights) drops back-to-back
    redundant LDWEIGHTS that Tile emits for matmuls sharing a
    stationary operand.

Stage 1 computes hT [H, T] = silu(w1 xT) * (w3 xT) per 128-row h-tile
(PSUM [128h, 512t] x2 token blocks, contraction over 16 D-tiles);
stage 2 computes out [T, D] db-outer/k-inner (PSUM [128t, 512d],
contraction over 11 H-tiles), each (tt, db) group draining its copy+DMA
while the next accumulates — the end-of-kernel tail is one dim-block.

_build_nc(reps=R, hw_loop=True) wraps the body in a device-side
tc.For_i loop for the timing protocol in test.py (constant NEFF size),
4 executions per iteration (software-pipelined: each later rep's input
DMAs overlap the previous rep's stage-2 under buffer-WAR gating, and
the separate 4+4 PSUM rotations per stage keep the PE seam stall-free
— HW-measured ~2.3 us/rep per halving of the barrier+head count).
"""

from contextlib import ExitStack

import numpy as np

import concourse.bass as bass
import concourse.mybir as mybir
import concourse.tile as tile
from concourse import bacc
from concourse.bass import ts
from concourse.bass_utils import run_bass_kernel_spmd

F32 = mybir.dt.float32
F16 = mybir.dt.float16

N_TOKENS = 8192
DIM = 2048
HIDDEN = 1408
N_EXPERTS = 8
CAP = N_TOKENS // N_EXPERTS  # 1024 tokens per core
P = 128
KD = DIM // P  # 16 contraction tiles, stage 1
KH = HIDDEN // P  # 11 contraction tiles, stage 2
TB = 512  # token-block (stage-1 moving free dim)
DB = 512  # dim-block (stage-2 moving free dim)
N_TB = CAP // TB  # 2
N_DB = DIM // DB  # 4
N_TT = CAP // P  # 8 token tiles (stage-2 stationary)

_CACHED_NC = None


def _build_nc(reps=1, hw_loop=False, unroll=4):
    nc = bacc.Bacc("TRN2", debug=False)
    xQ = nc.dram_tensor("xQ", [P, KD, CAP], F16, kind="ExternalInput").ap()
    w13Q = nc.dram_tensor("w13Q", [KH, P, KD, 2, P], F16, kind="ExternalInput").ap()
    w2Q = nc.dram_tensor("w2Q", [N_DB, P, KH, DB], F16, kind="ExternalInput").ap()
    out = nc.dram_tensor("out", [CAP, DIM], F32, kind="ExternalOutput").ap()

    with tile.TileContext(nc) as tc, ExitStack() as ctx:
        xpool = ctx.enter_context(tc.tile_pool(name="xpool", bufs=1))
        hpool = ctx.enter_context(tc.tile_pool(name="hpool", bufs=1))
        wpool = ctx.enter_context(tc.tile_pool(name="wpool", bufs=2))
        w2pool = ctx.enter_context(tc.tile_pool(name="w2pool", bufs=N_DB))
        tmppool = ctx.enter_context(tc.tile_pool(name="tmppool", bufs=3))
        opool = ctx.enter_context(tc.tile_pool(name="opool", bufs=4))
        # Separate 4-bank PSUM rotations per stage: stage-1 of rep i+1 then
        # reuses banks drained early in rep i's stage 1 (not rep i's last
        # stage-2 outputs), so back-to-back reps have no PSUM WAR stall at
        # the seam.
        ps1pool = ctx.enter_context(tc.tile_pool(name="ps1pool", bufs=4, space="PSUM"))
        ps2pool = ctx.enter_context(tc.tile_pool(name="ps2pool", bufs=4, space="PSUM"))

        def head_loads(streaming=True):
            # One rep's head inputs: x + the first two w13 pairs.
            # streaming=True (post-barrier pack leader): x arrives in
            # ramped k-batches and w13 pair0 in k-chunks so the cold PE
            # can start consuming ~4 us in; pair1 is dummy-gated on the
            # first x slice so its 1 MB transfer yields the SDMA device
            # head to the x stream. streaming=False (follower rep): the
            # loads are WAR-gated into the previous rep's stage-2 window,
            # so one bulk DMA each suffices.
            x_sb = xpool.tile([P, KD, CAP], F16)
            p0 = wpool.tile([P, KD, 2, P], F16, tag="w13")
            if streaming:
                ck = [(0, 2), (2, 2), (4, 4), (8, 8)]  # (start, len) k-chunks
                for c0, cl in ck:
                    nc.scalar.dma_start(
                        p0[:, c0 : c0 + cl], w13Q[0, :, c0 : c0 + cl]
                    )
                xb = [(0, 1), (1, 1), (2, 2), (4, 2), (6, 2), (8, 4), (12, 2), (14, 2)]
                for b0, bl in xb:
                    nc.sync.dma_start(x_sb[:, b0 : b0 + bl], xQ[:, b0 : b0 + bl])
                p1 = wpool.tile([P, KD, 2, P], F16, tag="w13")
                nc.vector.tensor_copy(p1[0:1, 0, 0, 0:1], x_sb[0:1, 0, 0:1])
                nc.scalar.dma_start(p1[:], w13Q[1])
            else:
                nc.scalar.dma_start(p0[:], w13Q[0])
                nc.sync.dma_start(x_sb[:], xQ[:])
                p1 = wpool.tile([P, KD, 2, P], F16, tag="w13")
                nc.scalar.dma_start(p1[:], w13Q[1])
            return x_sb, p0, p1

        def compute(x_sb, p0, p1, streaming=True):
            # hT tiles: [h-inner(part), h-tile, t]
            h_sb = hpool.tile([P, KH, CAP], F16)

            w13_t = [p0, p1]

            def load_pair(ht):
                # w1+w3 packed k-interleaved in one DRAM tensor: a single
                # dma_start per h-tile; the wpool WAR rotation gates pair
                # ht+1's transfer on iter ht-1 releasing the buffer.
                w13_sb = wpool.tile([P, KD, 2, P], F16, tag="w13")
                nc.scalar.dma_start(w13_sb[:], w13Q[ht])
                w13_t.append(w13_sb)

            # Stage 1: per h-tile, 4 accumulation groups each run as one
            # same-bank 16-k matmul run (PSUM bank switches between
            # consecutive matmuls cost ~20 ns; per-k LDWEIGHTS is free).
            for ht in range(KH):
                if 1 <= ht <= KH - 2:
                    load_pair(ht + 1)
                if ht == 0:
                    w2_t = []
                if ht in (2, 4, 6, 8):
                    # w2 tiles for stage 2: dummy-gated on the previous
                    # h-tile's output so the 1.44 MB transfers spread across
                    # mid-stage-1 instead of hoisting into the x window.
                    db = (ht - 2) // 2
                    w2_sb = w2pool.tile([P, KH, DB], F16, tag="w2")
                    nc.vector.tensor_copy(
                        w2_sb[0:1, 0, 0:1], h_sb[0:1, ht - 1, 0:1]
                    )
                    nc.scalar.dma_start(w2_sb[:], w2Q[db])
                    w2_t.append(w2_sb)
                w13_sb = w13_t[ht]
                # Same-PSUM-bank matmul runs: consecutive matmuls that hit
                # different PSUM banks pay ~20 ns each (HW-measured), so the
                # 4 accumulation groups run k-sequentially, one bank at a
                # time (LDWEIGHTS per k is free — hidden by pull-ahead).
                # ht=0 interleaves in 4-k blocks instead, so consumption
                # stays paced with the streaming x batches.
                kb = 4 if (ht == 0 and streaming) else KD
                ps1 = [
                    ps1pool.tile([P, TB], F32, tag="ps1", name=f"ps1_{ht}_{tb}")
                    for tb in range(N_TB)
                ]
                ps3 = [
                    ps1pool.tile([P, TB], F32, tag="ps1", name=f"ps3_{ht}_{tb}")
                    for tb in range(N_TB)
                ]
                for k0 in range(0, KD, kb):
                    for psg, w_idx, tb in (
                        (ps1[0], 0, 0), (ps3[0], 1, 0),
                        (ps1[1], 0, 1), (ps3[1], 1, 1),
                    ):
                        for k in range(k0, k0 + kb):
                            nc.tensor.matmul(
                                psg[:], w13_sb[:, k, w_idx],
                                x_sb[:, k, ts(tb, TB)],
                                start=(k == 0), stop=(k == KD - 1),
                            )
                for tb in range(N_TB):
                    sil = tmppool.tile([P, TB], F32, tag="sil")
                    nc.scalar.activation(
                        sil[:], ps1[tb][:], mybir.ActivationFunctionType.Silu
                    )
                    nc.vector.tensor_mul(
                        h_sb[:, ht, ts(tb, TB)], sil[:], ps3[tb][:]
                    )

            # Stage 2: out = hT.T @ w2.T — stationary hT token-tiles,
            # moving w2 dim-blocks. db-outer/k-inner: each (tt, db) PSUM
            # group is an 11-matmul same-bank run (no per-MM bank-switch
            # cost; the per-k stationary reloads are free), and each
            # group's copy+DMA drains while the next accumulates — which
            # also shrinks the end-of-kernel drain tail to one dim-block.
            for tt in range(N_TT):
                for db in range(N_DB):
                    ps2 = ps2pool.tile([P, DB], F32, tag="ps2",
                                       name=f"ps2_{tt}_{db}")
                    for k in range(KH):
                        nc.tensor.matmul(
                            ps2[:], h_sb[:, k, ts(tt, P)], w2_t[db][:, k],
                            start=(k == 0), stop=(k == KH - 1),
                        )
                    ot = opool.tile([P, DB], F32, tag="ot",
                                    name=f"ot_{tt}_{db}")
                    nc.vector.tensor_copy(ot[:], ps2[:])
                    nc.sync.dma_start(out[ts(tt, P), ts(db, DB)], ot[:])

        if hw_loop and reps > 1:
            # constant-size NEFF, `unroll` executions per For_i iteration:
            # follower reps' head loads (WAR-gated) transfer during the
            # previous rep's stage 2, so only the post-barrier pack
            # leader pays a (streamed, paced) DMA head.
            assert reps % unroll == 0, "reps must divide by unroll"
            with tc.For_i(0, reps // unroll):
                for j in range(unroll):
                    lead = j == 0
                    t = head_loads(streaming=lead)
                    compute(*t, streaming=lead)
        else:
            for _ in range(reps):
                t = head_loads(streaming=True)
                compute(*t, streaming=True)

    nc.compile()
    _dedup_ldweights(nc)
    return nc


def _dedup_ldweights(nc):
    """Drop back-to-back redundant LDWEIGHTS in the tile-lowered BIR.

    Tile's lowering emits one InstLdweights per InstMatmult even when
    consecutive matmuls share the stationary operand (the PE array keeps
    weights across matmuls with ldweights=false, so the reload is pure
    overhead — measured ~67-90 ns serialized per load). Runs after
    bacc.compile() (post move_matmul_waits_to_ldweights). A duplicate is
    removed only when (a) its operand AP is byte-identical to the
    surviving PE weight load with only Matmult(ldweights=false)/
    EventSemaphore in between on the PE stream, (b) it carries no
    semaphore updates, and (c) its waits are a subset of waits already
    observed on the PE stream since that load (sems are monotonic, so
    the condition is already guaranteed in the in-order PE stream).
    """
    import orjson

    j = orjson.loads(mybir.module_to_json_string(nc.m))
    removed = 0
    for fn in j["functions"]:
        for blk in fn["blocks"]:
            insts = blk.get("instructions", [])
            keep = []
            last_sig = None
            last_waits = []
            for inst in insts:
                if inst.get("engine") != "PE":
                    keep.append(inst)
                    continue
                op = inst.get("opcode")
                if op == "Ldweights":
                    sig = orjson.dumps(
                        [
                            inst.get("ins"),
                            inst.get("tile_position"),
                            inst.get("tile_size"),
                            inst.get("perf_mode"),
                            inst.get("is_transpose"),
                        ]
                    )
                    si = inst.get("sync_info") or {}
                    waits = [orjson.dumps(w) for w in (si.get("on_wait") or [])]
                    if (
                        sig == last_sig
                        and not si.get("on_update")
                        and all(w in last_waits for w in waits)
                    ):
                        removed += 1
                        continue
                    last_sig = sig
                    last_waits = waits
                elif op == "Matmult":
                    if inst.get("ldweights"):
                        last_sig = None  # self-loading matmul clobbers
                    else:
                        si = inst.get("sync_info") or {}
                        last_waits += [
                            orjson.dumps(w) for w in (si.get("on_wait") or [])
                        ]
                elif op != "EventSemaphore":
                    last_sig = None  # unknown PE op: be conservative
                keep.append(inst)
            blk["instructions"] = keep
    if removed:
        nc.m = mybir.module_from_json_bytes(orjson.dumps(j))
    return removed


def _strip_mm_incs_json(j, keep_every=10**9):
    """Strip per-Matmult semaphore increments from the lowered BIR.

    Tile emits `then_inc(PE_sem, 1)` on every Matmult; the EVT_SEM register
    write serializes on the PE at ~26 ns each (see tensor-engine tail
    model), which is pure overhead for the ~93% of matmuls no consumer
    waits on. This keeps an increment only where some wait threshold is
    crossed (plus every `keep_every`-th as padding) and renumbers every
    `sem-ge-imm` wait on those sems to the new cumulative counts. Release
    points are preserved instruction-exactly, so schedule semantics (and
    deadlock-freedom) are unchanged. UpdateValue must stay 1 (BIR ISA).
    """
    mm_sems = set()
    for fn in j["functions"]:
        for blk in fn["blocks"]:
            for inst in blk.get("instructions", []):
                if inst.get("engine") == "PE" and inst.get("opcode") == "Matmult":
                    si = inst.get("sync_info") or {}
                    for u in (si.get("on_update") or []):
                        if u.get("sync_type") == "semaphore":
                            assert u["update_mode"] == "sem-inc"
                            assert u["update_value"] == 1
                            mm_sems.add(u["id"])
    if not mm_sems:
        return 0, 0
    waited = {s: set() for s in mm_sems}
    for fn in j["functions"]:
        for blk in fn["blocks"]:
            for inst in blk.get("instructions", []):
                si = inst.get("sync_info") or {}
                for w in (si.get("on_wait") or []):
                    if w.get("id") in waited:
                        assert w["wait_mode"] == "sem-ge-imm", w
                        waited[w["id"]].add(w["wait_value"])
    val_map = {s: {0: 0} for s in mm_sems}
    cum_old = {s: 0 for s in mm_sems}
    cum_new = {s: 0 for s in mm_sems}
    affine = []  # barrier add/sub-imm updates carrying the per-iter total
    stripped = kept = 0
    for fn in j["functions"]:
        for blk in fn["blocks"]:
            for inst in blk.get("instructions", []):
                si = inst.get("sync_info") or {}
                upd = si.get("on_update") or []
                if not upd:
                    continue
                is_mm = (
                    inst.get("engine") == "PE"
                    and inst.get("opcode") == "Matmult"
                )
                new_upd = []
                for u in upd:
                    s = u.get("id")
                    if u.get("sync_type") != "semaphore" or s not in mm_sems:
                        new_upd.append(u)
                        continue
                    if not is_mm:
                        # For_i barrier bias/reset: carries the per-iteration
                        # inc total; rewritten to the new total below.
                        assert u["update_mode"] in ("sem-add-imm", "sem-sub-imm"), u
                        affine.append((s, u))
                        new_upd.append(u)
                        continue
                    cum_old[s] += 1
                    if cum_old[s] in waited[s] or cum_old[s] % keep_every == 0:
                        cum_new[s] += 1
                        val_map[s][cum_old[s]] = cum_new[s]
                        new_upd.append(u)
                        kept += 1
                    else:
                        stripped += 1
                si["on_update"] = new_upd
    for s, u in affine:
        assert u["update_value"] == cum_old[s], (u, cum_old[s])
        u["update_value"] = cum_new[s]
    for fn in j["functions"]:
        for blk in fn["blocks"]:
            for inst in blk.get("instructions", []):
                si = inst.get("sync_info") or {}
                for w in (si.get("on_wait") or []):
                    s = w.get("id")
                    if s in mm_sems:
                        w["wait_value"] = val_map[s][w["wait_value"]]
    return stripped, kept


def _get_nc():
    global _CACHED_NC
    if _CACHED_NC is None:
        _CACHED_NC = _build_nc()
    return _CACHED_NC


def _pack_inputs(x, w1, w2, w3, read_starts):
    """Per-core input dicts with DMA-optimal (partition-major) layouts."""
    in_maps = []
    for e in range(N_EXPERTS):
        s = int(read_starts[e])
        xe = x[s : s + CAP]  # [CAP, DIM]
        xQ = np.ascontiguousarray(
            xe.T.reshape(KD, P, CAP).transpose(1, 0, 2)
        ).astype(np.float16)
        w1Q = w1[e].T.reshape(KD, P, KH, P).transpose(2, 1, 0, 3)
        w3Q = w3[e].T.reshape(KD, P, KH, P).transpose(2, 1, 0, 3)
        # k-interleaved pack: [KH, P, KD, {w1,w3}, P] — one DMA per h-tile
        w13Q = np.ascontiguousarray(
            np.stack([w1Q, w3Q], axis=3)
        ).astype(np.float16)
        w2Q = np.ascontiguousarray(
            w2[e].T.reshape(KH, P, N_DB, DB).transpose(2, 1, 0, 3)
        ).astype(np.float16)
        in_maps.append({"xQ": xQ, "w13Q": w13Q, "w2Q": w2Q})
    return in_maps


def kernel(x, num_tokens_per_expert, w1, w2, w3):
    x = np.ascontiguousarray(np.asarray(x, dtype=np.float32))
    w1 = np.asarray(w1, dtype=np.float32)
    w2 = np.asarray(w2, dtype=np.float32)
    w3 = np.asarray(w3, dtype=np.float32)
    counts = np.asarray(num_tokens_per_expert).astype(np.int64)

    offsets = np.cumsum(counts)
    starts = offsets - counts
    # jax.lax.dynamic_slice clamps the read start so the slice is in-bounds.
    read_starts = np.clip(starts, 0, N_TOKENS - CAP)

    in_maps = _pack_inputs(x, w1, w2, w3, read_starts)
    nc = _get_nc()
    res = run_bass_kernel_spmd(nc, in_maps, core_ids=list(range(N_EXPERTS)))
    ye = [res.results[e]["out"] for e in range(N_EXPERTS)]

    if np.all(counts == CAP):
        # balanced routing: per-expert tiles are disjoint and exactly cover x
        return np.concatenate(ye, axis=0)

    # general case: mask invalid slots, scatter-add to clipped positions
    y = np.zeros((N_TOKENS, DIM), np.float32)
    slot = np.arange(CAP)
    for e in range(N_EXPERTS):
        valid = slot < counts[e]
        pos = np.clip(starts[e] + slot, 0, N_TOKENS - 1)
        np.add.at(y, pos, np.where(valid[:, None], ye[e], 0.0))
    return y

